# revision 10
# baseline (speedup 1.0000x reference)
"""Trainium2 Bass kernel for an input-attention LSTM encoder (DA-RNN style).

Reference semantics (per batch b, timestep t):
    s      = h @ w_h + c @ w_c                      # scalar per batch
    logits = s + precomp[b, :] + b_attn             # precomp = einsum('btn,t->bn', x, w_x)
    attn   = softmax(logits)                        # over N drivers
    wx     = attn * x[:, t, :]
    gates  = wx @ w_ih.T + h @ w_hh.T + (b_ih + b_hh)   # i, f, g, o
    c'     = sigmoid(f) * c + sigmoid(i) * tanh(g)
    h'     = sigmoid(o) * tanh(c')
Outputs: input_weighted = stack_t(wx)  [B, T, N]
         input_encoded  = stack_t(h')  [B, T, H]

Mapping onto one NeuronCore (data-parallel over batch, 8 cores, 256 rows each,
processed as 2 independent 128-row chains so the engines pipeline):
  - All tensors live in natural [batch(partition), feature(free)] layout.
  - sigma(x) = 0.5 + 0.5*tanh(x/2), and the g-gate's weight rows are pre-doubled
    host-side, so ONE ScalarE tanh(0.5*z) over all 4H gate columns produces
    t_i, t_f, t_g(=tanh(g)), t_o. The table set is exp_and_others (exp + tanh)
    so there is no per-step activation-table switch.
  - cell update via fused VectorE ops:
      A  = (0.5*t_f + 0.5) * c        (affine_mul_reduce)      = sigmoid(f)*c
      B1 = (t_i + 1) * t_g            (scalar_tensor_tensor)
      c' = 0.5*B1 + A                 (scalar_tensor_tensor)
      h' = (0.5*t_o + 0.5) * tanh(c') (affine_mul_reduce)
  - s_c = sum_k c'[b,k]*w_c[k] via scalar_tensor_tensor's fused accum_out;
    s_h via two tiny PE matmuls against the transposed hidden state.
    (tensor_tensor_reduce faults at runtime on this stack; STT+accum_out is
    the equivalent fusion on a known-good ISA op. GpSimd lacks the
    TensorScalarPtr ISA op entirely, so everything elementwise runs DVE/ACT.)
  - softmax: ScalarE exp(precomp + s) with the per-batch bias slot carrying s
    and the fused accum_out producing the denominator; then
    wx = (exp * recip) * x in one scalar_tensor_tensor.
  - gates: PE matmuls with the *weights* as the wide moving operand
    (rhs [K,512]), activations transposed on PE as the stationary operand.
    The gate bias is injected into PSUM with a K=1 ones-row matmul.
"""

import os
import sys

for _p in ("/opt/trn_rl_repo",):
    if _p not in sys.path and os.path.isdir(_p):
        sys.path.insert(0, _p)

import numpy as np

import concourse.bass as bass
import concourse.tile as tile
from concourse import bacc, mybir
from concourse.bass_utils import run_bass_kernel_spmd

F32 = mybir.dt.float32
F16 = mybir.dt.float16
ALU = mybir.AluOpType
AF = mybir.ActivationFunctionType

B, T, N, H = 2048, 128, 128, 256
G4 = 4 * H
NCORES = 8
B_LOC = B // NCORES  # 256
CHAINS = 2


def emit_kernel(tc, io, B_ch, T_steps, chains):
    """Emit the per-core program. io: dict of DRAM APs/handles."""
    nc = tc.nc
    ctx = tc.ctx  # TileContext owns an ExitStack; use our own instead
    from contextlib import ExitStack

    es = ExitStack()

    const = es.enter_context(tc.tile_pool(name="const", bufs=1))
    xpool = es.enter_context(tc.tile_pool(name="xres", bufs=1))
    state = es.enter_context(tc.tile_pool(name="state", bufs=2))
    work = es.enter_context(tc.tile_pool(name="work", bufs=2))
    small = es.enter_context(tc.tile_pool(name="small", bufs=3))
    psum_g = es.enter_context(tc.tile_pool(name="psum_g", bufs=1, space="PSUM"))
    psum_m = es.enter_context(tc.tile_pool(name="psum_m", bufs=1, space="PSUM"))

    # ---- constants into SBUF ----
    Wm_sb = const.tile([N, G4], F16, tag="Wm")
    nc.sync.dma_start(Wm_sb[:], io["Wm"][:])
    Um_sb = const.tile([128, 2, G4], F16, tag="Um")
    for k in range(2):
        nc.sync.dma_start(Um_sb[:, k, :], io["Um"][k * 128:(k + 1) * 128, :])
    brow_sb = const.tile([1, G4], F16, tag="brow")
    nc.sync.dma_start(brow_sb[:], io["brow"][:])
    ones_sb = const.tile([1, 128], F16, tag="ones")
    nc.sync.dma_start(ones_sb[:], io["ones"][:])
    wh_sb = const.tile([128, 2], F16, tag="wh")
    nc.sync.dma_start(wh_sb[:], io["wh"][:])
    wcb_sb = const.tile([128, H], F32, tag="wcb")
    nc.sync.dma_start(wcb_sb[:], io["wcb"][:])
    wxt_sb = const.tile([128, T_steps], F32, tag="wxt")
    nc.sync.dma_start(wxt_sb[:], io["wxt"][:])
    battn_sb = const.tile([128, 1], F32, tag="battn")
    nc.sync.dma_start(battn_sb[:], io["battn"][:])

    x_res = []
    precomp = []
    for c in range(chains):
        b0 = c * B_ch
        xr = xpool.tile([B_ch, T_steps, N], F32, tag=f"x{c}")
        # chunked loads so multiple DMA queues run in parallel
        tchunk = max(1, T_steps // 8)
        for t0 in range(0, T_steps, tchunk):
            t1 = min(t0 + tchunk, T_steps)
            nc.sync.dma_start(xr[:, t0:t1, :], io["x"][b0:b0 + B_ch, t0:t1, :])
        x_res.append(xr)
        precomp.append(const.tile([B_ch, N], F32, tag=f"precomp{c}", name=f"precomp{c}"))

    # ---- precomp[b, n] = sum_t x[b, t, n] * w_x[t]  (split across DVE/GpSimd) ----
    for c in range(chains):
        acc_v = work.tile([B_ch, N], F32, tag=f"accv{c}")
        acc_g = work.tile([B_ch, N], F32, tag=f"accg{c}")
        nc.vector.memset(acc_v[:], 0.0)
        nc.vector.memset(acc_g[:], 0.0)
        half = T_steps // 2
        for t in range(T_steps):
            acc = acc_v if t < half else acc_g
            nc.vector.scalar_tensor_tensor(
                acc[:], x_res[c][:, t, :], wxt_sb[:B_ch, t:t + 1], acc[:],
                ALU.mult, ALU.add)
        nc.vector.tensor_add(precomp[c][:], acc_v[:], acc_g[:])

    # ---- per-chain loop state ----
    h_t, c_t, hT_t, s_t = [], [], [], []
    for c in range(chains):
        ht = state.tile([B_ch, H], F32, tag=f"h{c}")
        ct = state.tile([B_ch, H], F32, tag=f"c{c}")
        hTt = state.tile([128, 2, B_ch], F16, tag=f"hT{c}")
        st = small.tile([B_ch, 1], F32, tag=f"s{c}")
        nc.vector.memset(ht[:], 0.0)
        nc.vector.memset(ct[:], 0.0)
        nc.vector.memset(hTt[:], 0.0)
        nc.vector.tensor_copy(st[:], battn_sb[:B_ch, :])  # s(0) = b_attn
        h_t.append(ht); c_t.append(ct); hT_t.append(hTt); s_t.append(st)

    gates_ps = [psum_g.tile([B_ch, G4], F32, tag=f"g{c}", name=f"gps{c}") for c in range(chains)]
    misc_ps = [psum_m.tile([B_ch, 1], F32, tag=f"m{c}", name=f"mps{c}") for c in range(chains)]

    for t in range(T_steps):
        for c in range(chains):
            b0 = c * B_ch
            # ---- attention / weighted input ----
            expv = work.tile([B_ch, N], F32, tag=f"expv{c}")
            denom = small.tile([B_ch, 1], F32, tag=f"den{c}")
            nc.scalar.activation(expv[:], precomp[c][:], AF.Exp,
                                 bias=s_t[c][:], scale=1.0, accum_out=denom[:])
            rec = small.tile([B_ch, 1], F32, tag=f"rec{c}")
            nc.vector.reciprocal(rec[:], denom[:])
            wx = work.tile([B_ch, N], F32, tag=f"wx{c}")
            nc.vector.scalar_tensor_tensor(
                wx[:], expv[:], rec[:], x_res[c][:, t, :], ALU.mult, ALU.mult)
            nc.sync.dma_start(io["out_w"][b0:b0 + B_ch, t, :], wx[:])
            # wxT for the gates matmul (fp16 cast + xbar DMA transpose)
            wx16 = work.tile([B_ch, N], F16, tag=f"wx16{c}")
            nc.vector.tensor_copy(wx16[:], wx[:])
            wxT = work.tile([N, B_ch], F16, tag=f"wxT{c}")
            nc.sync.dma_start_transpose(wxT[:], wx16[:])

            # ---- gates = wx@W^T + h@U^T + b  (PSUM [B_ch, 4H]) ----
            gp = gates_ps[c]
            for j in range(2):
                sl = slice(j * 512, (j + 1) * 512)
                nc.tensor.matmul(gp[:, sl], wxT[:], Wm_sb[:, sl],
                                 start=True, stop=False)
                nc.tensor.matmul(gp[:, sl], hT_t[c][:, 0, :], Um_sb[:, 0, sl],
                                 start=False, stop=False)
                nc.tensor.matmul(gp[:, sl], hT_t[c][:, 1, :], Um_sb[:, 1, sl],
                                 start=False, stop=False)
                nc.tensor.matmul(gp[:, sl], ones_sb[:, :B_ch], brow_sb[:, sl],
                                 start=False, stop=True)
            tIFOG = work.tile([B_ch, G4], F32, tag=f"tIFOG{c}")
            nc.scalar.activation(tIFOG[:], gp[:], AF.Tanh, scale=0.5)

            ti = tIFOG[:, 0:H]
            tf = tIFOG[:, H:2 * H]
            tg = tIFOG[:, 2 * H:3 * H]
            to = tIFOG[:, 3 * H:4 * H]

            # ---- cell/hidden update ----
            dmy1 = small.tile([B_ch, 1], F32, tag=f"dm1{c}")
            A = work.tile([B_ch, H], F32, tag=f"A{c}")
            nc.vector.affine_mul_reduce(A[:], dmy1[:], tf, c_t[c][:], 0.5, 0.5)
            B1 = work.tile([B_ch, H], F32, tag=f"B1{c}")
            nc.vector.scalar_tensor_tensor(B1[:], ti, 1.0, tg, ALU.add, ALU.mult)
            c_new = state.tile([B_ch, H], F32, tag=f"c{c}")
            nc.vector.scalar_tensor_tensor(c_new[:], B1[:], 0.5, A[:],
                                           ALU.mult, ALU.add)
            tcn = work.tile([B_ch, H], F32, tag=f"tc{c}")
            nc.scalar.activation(tcn[:], c_new[:], AF.Tanh, scale=1.0)
            dmy2 = small.tile([B_ch, 1], F32, tag=f"dm2{c}")
            h_new = state.tile([B_ch, H], F32, tag=f"h{c}")
            nc.vector.affine_mul_reduce(h_new[:], dmy2[:], to, tcn[:], 0.5, 0.5)
            nc.sync.dma_start(io["out_h"][b0:b0 + B_ch, t, :], h_new[:])

            c_t[c] = c_new
            h_t[c] = h_new

            if t == T_steps - 1:
                continue

            # ---- next-step s and transposed h ----
            h16 = work.tile([B_ch, H], F16, tag=f"h16{c}")
            nc.vector.tensor_copy(h16[:], h_new[:])
            hT_new = state.tile([128, 2, B_ch], F16, tag=f"hT{c}")
            for k in range(2):
                nc.sync.dma_start_transpose(hT_new[:, k, :],
                                            h16[:, k * 128:(k + 1) * 128])
            for k in range(2):
                nc.tensor.matmul(misc_ps[c][:, 0:1],
                                 hT_new[:, k, :], wh_sb[:, k:k + 1],
                                 start=(k == 0), stop=(k == 1))
            hT_t[c] = hT_new
            scr = work.tile([B_ch, H], F32, tag=f"scr{c}")
            sc = small.tile([B_ch, 1], F32, tag=f"sc{c}")
            nc.vector.scalar_tensor_tensor(
                scr[:], c_new[:], 1.0, wcb_sb[:B_ch, :], ALU.mult, ALU.mult,
                accum_out=sc[:])
            s_new = small.tile([B_ch, 1], F32, tag=f"s{c}")
            nc.vector.scalar_tensor_tensor(
                s_new[:], misc_ps[c][:, 0:1],
                battn_sb[:B_ch, :], sc[:], ALU.add, ALU.add)
            s_t[c] = s_new

    es.close()


def prep_consts(w_ih, w_hh, b_ih, b_hh, w_attn, b_attn, T_steps=T):
    """Host-side weight layout prep (numpy)."""
    Hh = H
    Wm = np.ascontiguousarray(w_ih.T).astype(np.float32).copy()   # [N, 4H]
    Um = np.ascontiguousarray(w_hh.T).astype(np.float32).copy()   # [H, 4H]
    brow = (b_ih + b_hh).astype(np.float32)[None, :].copy()       # [1, 4H]
    gsl = slice(2 * Hh, 3 * Hh)
    Wm[:, gsl] *= 2.0
    Um[:, gsl] *= 2.0
    brow[:, gsl] *= 2.0
    wh = np.ascontiguousarray(w_attn[:Hh].reshape(2, 128).T).astype(np.float16)  # [128,2]
    wcb = np.broadcast_to(w_attn[Hh:2 * Hh], (128, Hh)).astype(np.float32).copy()
    wxt = np.broadcast_to(w_attn[2 * Hh:2 * Hh + T_steps], (128, T_steps)).astype(np.float32).copy()
    battn = np.broadcast_to(b_attn.reshape(1, 1), (128, 1)).astype(np.float32).copy()
    ident = np.eye(128, dtype=np.float32)
    ones = np.ones((1, 128), np.float32)
    return dict(Wm=Wm.astype(np.float16), Um=Um.astype(np.float16),
                brow=brow.astype(np.float16), wh=wh, wcb=wcb, wxt=wxt,
                battn=battn, ident=ident, ones=ones.astype(np.float16))


_PROGRAM_CACHE = {}


def build_program(B_ch=B_LOC // CHAINS, T_steps=T, chains=CHAINS):
    key = (B_ch, T_steps, chains)
    if key in _PROGRAM_CACHE:
        return _PROGRAM_CACHE[key]
    nc = bacc.Bacc("TRN2", target_bir_lowering=False, debug=False)
    B_core = B_ch * chains
    io = {
        "x": nc.declare_dram_parameter("x", [B_core, T_steps, N], F32, isOutput=False),
        "Wm": nc.declare_dram_parameter("Wm", [N, G4], F16, isOutput=False),
        "Um": nc.declare_dram_parameter("Um", [H, G4], F16, isOutput=False),
        "brow": nc.declare_dram_parameter("brow", [1, G4], F16, isOutput=False),
        "wh": nc.declare_dram_parameter("wh", [128, 2], F16, isOutput=False),
        "wcb": nc.declare_dram_parameter("wcb", [128, H], F32, isOutput=False),
        "wxt": nc.declare_dram_parameter("wxt", [128, T_steps], F32, isOutput=False),
        "battn": nc.declare_dram_parameter("battn", [128, 1], F32, isOutput=False),
        "ident": nc.declare_dram_parameter("ident", [128, 128], F32, isOutput=False),
        "ones": nc.declare_dram_parameter("ones", [1, 128], F16, isOutput=False),
        "out_w": nc.declare_dram_parameter("out_w", [B_core, T_steps, N], F32, isOutput=True),
        "out_h": nc.declare_dram_parameter("out_h", [B_core, T_steps, H], F32, isOutput=True),
    }
    with tile.TileContext(nc) as tc:
        emit_kernel(tc, io, B_ch, T_steps, chains)
    nc.compile()
    _PROGRAM_CACHE[key] = nc
    return nc


LAST_RESULTS = None


def kernel(input_data, w_ih, w_hh, b_ih, b_hh, w_attn, b_attn):
    global LAST_RESULTS
    x = np.ascontiguousarray(np.asarray(input_data, dtype=np.float32))
    consts = prep_consts(np.asarray(w_ih), np.asarray(w_hh), np.asarray(b_ih),
                         np.asarray(b_hh), np.asarray(w_attn), np.asarray(b_attn))
    nc = build_program()
    in_maps = []
    for core in range(NCORES):
        m = dict(consts)
        m["x"] = x[core * B_LOC:(core + 1) * B_LOC]
        in_maps.append(m)
    trace = bool(int(os.environ.get("TRN_KERNEL_TRACE", "0")))
    if trace:
        try:
            from antenv.axon_hooks import get_axon_ntff_profile_hook
            trace = get_axon_ntff_profile_hook() is not None
        except ImportError:
            trace = False
    res = run_bass_kernel_spmd(nc, in_maps, list(range(NCORES)), trace=trace)
    LAST_RESULTS = res
    out_w = np.concatenate([res.results[i]["out_w"] for i in range(NCORES)], axis=0)
    out_h = np.concatenate([res.results[i]["out_h"] for i in range(NCORES)], axis=0)
    return out_w, out_h


# revision 11
# speedup vs baseline: 1.5318x; 1.5318x over previous
"""Trainium2 Bass kernel for an input-attention LSTM encoder (DA-RNN style).

Reference semantics (per batch b, timestep t):
    s      = h @ w_h + c @ w_c                      # scalar per batch
    logits = s + precomp[b, :] + b_attn             # precomp = einsum('btn,t->bn', x, w_x)
    attn   = softmax(logits)                        # over N drivers
    wx     = attn * x[:, t, :]
    gates  = wx @ w_ih.T + h @ w_hh.T + (b_ih + b_hh)   # i, f, g, o
    c'     = sigmoid(f) * c + sigmoid(i) * tanh(g)
    h'     = sigmoid(o) * tanh(c')
Outputs: input_weighted = stack_t(wx)  [B, T, N]
         input_encoded  = stack_t(h')  [B, T, H]

Mapping onto one NeuronCore (data-parallel over batch, 8 cores, 256 rows each,
processed as 2 independent 128-row chains so the engines pipeline):
  - All tensors live in natural [batch(partition), feature(free)] layout.
  - sigma(x) = 0.5 + 0.5*tanh(x/2), and the g-gate's weight rows are pre-doubled
    host-side, so ONE ScalarE tanh(0.5*z) over all 4H gate columns produces
    t_i, t_f, t_g(=tanh(g)), t_o. The table set is exp_and_others (exp + tanh)
    so there is no per-step activation-table switch.
  - cell update via fused VectorE ops:
      A  = (0.5*t_f + 0.5) * c        (affine_mul_reduce)      = sigmoid(f)*c
      B1 = (t_i + 1) * t_g            (scalar_tensor_tensor)
      c' = 0.5*B1 + A                 (scalar_tensor_tensor)
      h' = (0.5*t_o + 0.5) * tanh(c') (affine_mul_reduce)
  - s_c = sum_k c'[b,k]*w_c[k] via scalar_tensor_tensor's fused accum_out;
    s_h via two tiny PE matmuls against the transposed hidden state.
    (tensor_tensor_reduce faults at runtime on this stack; STT+accum_out is
    the equivalent fusion on a known-good ISA op. GpSimd lacks the
    TensorScalarPtr ISA op entirely, so everything elementwise runs DVE/ACT.)
  - softmax: ScalarE exp(precomp + s) with the per-batch bias slot carrying s
    and the fused accum_out producing the denominator; then
    wx = (exp * recip) * x in one scalar_tensor_tensor.
  - gates: PE matmuls with the *weights* as the wide moving operand
    (rhs [K,512]), activations transposed on PE as the stationary operand.
    The gate bias is injected into PSUM with a K=1 ones-row matmul.
"""

import os
import sys

for _p in ("/opt/trn_rl_repo",):
    if _p not in sys.path and os.path.isdir(_p):
        sys.path.insert(0, _p)

import numpy as np

import concourse.bass as bass
import concourse.tile as tile
from concourse import bacc, mybir
from concourse.bass_utils import run_bass_kernel_spmd

F32 = mybir.dt.float32
F16 = mybir.dt.float16
ALU = mybir.AluOpType
AF = mybir.ActivationFunctionType

B, T, N, H = 2048, 128, 128, 256
G4 = 4 * H
NCORES = 8
B_LOC = B // NCORES  # 256
CHAINS = 2


def emit_kernel(tc, io, B_ch, T_steps, chains):
    """Emit the per-core program. io: dict of DRAM APs/handles."""
    nc = tc.nc
    ctx = tc.ctx  # TileContext owns an ExitStack; use our own instead
    from contextlib import ExitStack

    es = ExitStack()

    const = es.enter_context(tc.tile_pool(name="const", bufs=1))
    xpool = es.enter_context(tc.tile_pool(name="xres", bufs=1))
    state = es.enter_context(tc.tile_pool(name="state", bufs=2))
    work = es.enter_context(tc.tile_pool(name="work", bufs=2))
    small = es.enter_context(tc.tile_pool(name="small", bufs=3))
    psum_g = es.enter_context(tc.tile_pool(name="psum_g", bufs=1, space="PSUM"))

    # ---- constants into SBUF ----
    Wm_sb = const.tile([N, G4], F16, tag="Wm")
    nc.sync.dma_start(Wm_sb[:], io["Wm"][:])
    Um_sb = const.tile([128, 2, G4], F16, tag="Um")
    for k in range(2):
        nc.sync.dma_start(Um_sb[:, k, :], io["Um"][k * 128:(k + 1) * 128, :])
    brow_sb = const.tile([1, G4], F16, tag="brow")
    nc.sync.dma_start(brow_sb[:], io["brow"][:])
    ones_sb = const.tile([1, 128], F16, tag="ones")
    nc.sync.dma_start(ones_sb[:], io["ones"][:])
    wcb_sb = const.tile([128, H], F32, tag="wcb")
    nc.sync.dma_start(wcb_sb[:], io["wcb"][:])
    whb_sb = const.tile([128, H], F32, tag="whb")
    nc.sync.dma_start(whb_sb[:], io["whb"][:])
    wxt_sb = const.tile([128, T_steps], F32, tag="wxt")
    nc.sync.dma_start(wxt_sb[:], io["wxt"][:])
    battn_sb = const.tile([128, 1], F32, tag="battn")
    nc.sync.dma_start(battn_sb[:], io["battn"][:])

    x_res = []
    precomp = []
    for c in range(chains):
        b0 = c * B_ch
        xr = xpool.tile([B_ch, T_steps, N], F32, tag=f"x{c}")
        # chunked loads so multiple DMA queues run in parallel
        tchunk = max(1, T_steps // 8)
        for t0 in range(0, T_steps, tchunk):
            t1 = min(t0 + tchunk, T_steps)
            nc.sync.dma_start(xr[:, t0:t1, :], io["x"][b0:b0 + B_ch, t0:t1, :])
        x_res.append(xr)
        precomp.append(const.tile([B_ch, N], F32, tag=f"precomp{c}", name=f"precomp{c}"))

    # ---- precomp[b, n] = sum_t x[b, t, n] * w_x[t]  (split across DVE/GpSimd) ----
    for c in range(chains):
        acc_v = work.tile([B_ch, N], F32, tag=f"accv{c}")
        acc_g = work.tile([B_ch, N], F32, tag=f"accg{c}")
        nc.vector.memset(acc_v[:], 0.0)
        nc.vector.memset(acc_g[:], 0.0)
        half = T_steps // 2
        for t in range(T_steps):
            acc = acc_v if t < half else acc_g
            nc.vector.scalar_tensor_tensor(
                acc[:], x_res[c][:, t, :], wxt_sb[:B_ch, t:t + 1], acc[:],
                ALU.mult, ALU.add)
        nc.vector.tensor_add(precomp[c][:], acc_v[:], acc_g[:])

    # ---- per-chain loop state ----
    h_t, c_t, hT_t, s_t = [], [], [], []
    for c in range(chains):
        ht = state.tile([B_ch, H], F32, tag=f"h{c}")
        ct = state.tile([B_ch, H], F32, tag=f"c{c}")
        hTt = state.tile([128, 2, B_ch], F16, tag=f"hT{c}")
        st = small.tile([B_ch, 1], F32, tag=f"s{c}")
        nc.vector.memset(ht[:], 0.0)
        nc.vector.memset(ct[:], 0.0)
        nc.vector.memset(hTt[:], 0.0)
        nc.vector.tensor_copy(st[:], battn_sb[:B_ch, :])  # s(0) = b_attn
        h_t.append(ht); c_t.append(ct); hT_t.append(hTt); s_t.append(st)

    gates_ps = [psum_g.tile([B_ch, G4], F32, tag=f"g{c}", name=f"gps{c}") for c in range(chains)]

    for t in range(T_steps):
        for c in range(chains):
            b0 = c * B_ch
            # ---- attention / weighted input ----
            expv = work.tile([B_ch, N], F32, tag=f"expv{c}")
            denom = small.tile([B_ch, 1], F32, tag=f"den{c}")
            nc.scalar.activation(expv[:], precomp[c][:], AF.Exp,
                                 bias=s_t[c][:], scale=1.0, accum_out=denom[:])
            rec = small.tile([B_ch, 1], F32, tag=f"rec{c}")
            nc.vector.reciprocal(rec[:], denom[:])
            wx = work.tile([B_ch, N], F32, tag=f"wx{c}")
            nc.vector.scalar_tensor_tensor(
                wx[:], expv[:], rec[:], x_res[c][:, t, :], ALU.mult, ALU.mult)
            nc.sync.dma_start(io["out_w"][b0:b0 + B_ch, t, :], wx[:])
            # wxT for the gates matmul (fp16 cast + xbar DMA transpose)
            wx16 = work.tile([B_ch, N], F16, tag=f"wx16{c}")
            nc.vector.tensor_copy(wx16[:], wx[:])
            wxT = work.tile([N, B_ch], F16, tag=f"wxT{c}")
            nc.sync.dma_start_transpose(wxT[:], wx16[:])

            # ---- gates = wx@W^T + h@U^T + b  (PSUM [B_ch, 4H]) ----
            gp = gates_ps[c]
            for j in range(2):
                sl = slice(j * 512, (j + 1) * 512)
                nc.tensor.matmul(gp[:, sl], wxT[:], Wm_sb[:, sl],
                                 start=True, stop=False)
                nc.tensor.matmul(gp[:, sl], hT_t[c][:, 0, :], Um_sb[:, 0, sl],
                                 start=False, stop=False)
                nc.tensor.matmul(gp[:, sl], hT_t[c][:, 1, :], Um_sb[:, 1, sl],
                                 start=False, stop=False)
                nc.tensor.matmul(gp[:, sl], ones_sb[:, :B_ch], brow_sb[:, sl],
                                 start=False, stop=True)
            tIFOG = work.tile([B_ch, G4], F32, tag=f"tIFOG{c}")
            nc.scalar.activation(tIFOG[:], gp[:], AF.Tanh, scale=0.5)

            ti = tIFOG[:, 0:H]
            tf = tIFOG[:, H:2 * H]
            tg = tIFOG[:, 2 * H:3 * H]
            to = tIFOG[:, 3 * H:4 * H]

            # ---- cell/hidden update ----
            dmy1 = small.tile([B_ch, 1], F32, tag=f"dm1{c}")
            A = work.tile([B_ch, H], F32, tag=f"A{c}")
            nc.vector.affine_mul_reduce(A[:], dmy1[:], tf, c_t[c][:], 0.5, 0.5)
            B1 = work.tile([B_ch, H], F32, tag=f"B1{c}")
            nc.vector.scalar_tensor_tensor(B1[:], ti, 1.0, tg, ALU.add, ALU.mult)
            c_new = state.tile([B_ch, H], F32, tag=f"c{c}")
            nc.vector.scalar_tensor_tensor(c_new[:], B1[:], 0.5, A[:],
                                           ALU.mult, ALU.add)
            tcn = work.tile([B_ch, H], F32, tag=f"tc{c}")
            nc.scalar.activation(tcn[:], c_new[:], AF.Tanh, scale=1.0)
            dmy2 = small.tile([B_ch, 1], F32, tag=f"dm2{c}")
            h_new = state.tile([B_ch, H], F32, tag=f"h{c}")
            nc.vector.affine_mul_reduce(h_new[:], dmy2[:], to, tcn[:], 0.5, 0.5)
            nc.sync.dma_start(io["out_h"][b0:b0 + B_ch, t, :], h_new[:])

            c_t[c] = c_new
            h_t[c] = h_new

            if t == T_steps - 1:
                continue

            # ---- next-step s and transposed h ----
            h16 = work.tile([B_ch, H], F16, tag=f"h16{c}")
            nc.vector.tensor_copy(h16[:], h_new[:])
            hT_new = state.tile([128, 2, B_ch], F16, tag=f"hT{c}")
            for k in range(2):
                nc.sync.dma_start_transpose(hT_new[:, k, :],
                                            h16[:, k * 128:(k + 1) * 128])
            hT_t[c] = hT_new
            scrh = work.tile([B_ch, H], F32, tag=f"scrh{c}")
            sh = small.tile([B_ch, 1], F32, tag=f"sh{c}")
            nc.vector.scalar_tensor_tensor(
                scrh[:], h_new[:], 1.0, whb_sb[:B_ch, :], ALU.mult, ALU.mult,
                accum_out=sh[:])
            scr = work.tile([B_ch, H], F32, tag=f"scr{c}")
            sc = small.tile([B_ch, 1], F32, tag=f"sc{c}")
            nc.vector.scalar_tensor_tensor(
                scr[:], c_new[:], 1.0, wcb_sb[:B_ch, :], ALU.mult, ALU.mult,
                accum_out=sc[:])
            s_new = small.tile([B_ch, 1], F32, tag=f"s{c}")
            nc.vector.scalar_tensor_tensor(
                s_new[:], sh[:],
                battn_sb[:B_ch, :], sc[:], ALU.add, ALU.add)
            s_t[c] = s_new

    es.close()


def prep_consts(w_ih, w_hh, b_ih, b_hh, w_attn, b_attn, T_steps=T):
    """Host-side weight layout prep (numpy)."""
    Hh = H
    Wm = np.ascontiguousarray(w_ih.T).astype(np.float32).copy()   # [N, 4H]
    Um = np.ascontiguousarray(w_hh.T).astype(np.float32).copy()   # [H, 4H]
    brow = (b_ih + b_hh).astype(np.float32)[None, :].copy()       # [1, 4H]
    gsl = slice(2 * Hh, 3 * Hh)
    Wm[:, gsl] *= 2.0
    Um[:, gsl] *= 2.0
    brow[:, gsl] *= 2.0
    wh = np.ascontiguousarray(w_attn[:Hh].reshape(2, 128).T).astype(np.float16)  # [128,2]
    wcb = np.broadcast_to(w_attn[Hh:2 * Hh], (128, Hh)).astype(np.float32).copy()
    whb = np.broadcast_to(w_attn[:Hh], (128, Hh)).astype(np.float32).copy()
    wxt = np.broadcast_to(w_attn[2 * Hh:2 * Hh + T_steps], (128, T_steps)).astype(np.float32).copy()
    battn = np.broadcast_to(b_attn.reshape(1, 1), (128, 1)).astype(np.float32).copy()
    ident = np.eye(128, dtype=np.float32)
    ones = np.ones((1, 128), np.float32)
    return dict(Wm=Wm.astype(np.float16), Um=Um.astype(np.float16),
                brow=brow.astype(np.float16), wh=wh, wcb=wcb, whb=whb,
                wxt=wxt, battn=battn, ident=ident,
                ones=ones.astype(np.float16))


_PROGRAM_CACHE = {}


def build_program(B_ch=B_LOC // CHAINS, T_steps=T, chains=CHAINS):
    key = (B_ch, T_steps, chains)
    if key in _PROGRAM_CACHE:
        return _PROGRAM_CACHE[key]
    nc = bacc.Bacc("TRN2", target_bir_lowering=False, debug=False)
    B_core = B_ch * chains
    io = {
        "x": nc.declare_dram_parameter("x", [B_core, T_steps, N], F32, isOutput=False),
        "Wm": nc.declare_dram_parameter("Wm", [N, G4], F16, isOutput=False),
        "Um": nc.declare_dram_parameter("Um", [H, G4], F16, isOutput=False),
        "brow": nc.declare_dram_parameter("brow", [1, G4], F16, isOutput=False),
        "wh": nc.declare_dram_parameter("wh", [128, 2], F16, isOutput=False),
        "wcb": nc.declare_dram_parameter("wcb", [128, H], F32, isOutput=False),
        "whb": nc.declare_dram_parameter("whb", [128, H], F32, isOutput=False),
        "wxt": nc.declare_dram_parameter("wxt", [128, T_steps], F32, isOutput=False),
        "battn": nc.declare_dram_parameter("battn", [128, 1], F32, isOutput=False),
        "ident": nc.declare_dram_parameter("ident", [128, 128], F32, isOutput=False),
        "ones": nc.declare_dram_parameter("ones", [1, 128], F16, isOutput=False),
        "out_w": nc.declare_dram_parameter("out_w", [B_core, T_steps, N], F32, isOutput=True),
        "out_h": nc.declare_dram_parameter("out_h", [B_core, T_steps, H], F32, isOutput=True),
    }
    with tile.TileContext(nc) as tc:
        emit_kernel(tc, io, B_ch, T_steps, chains)
    nc.compile()
    _PROGRAM_CACHE[key] = nc
    return nc


LAST_RESULTS = None


def kernel(input_data, w_ih, w_hh, b_ih, b_hh, w_attn, b_attn):
    global LAST_RESULTS
    x = np.ascontiguousarray(np.asarray(input_data, dtype=np.float32))
    consts = prep_consts(np.asarray(w_ih), np.asarray(w_hh), np.asarray(b_ih),
                         np.asarray(b_hh), np.asarray(w_attn), np.asarray(b_attn))
    nc = build_program()
    in_maps = []
    for core in range(NCORES):
        m = dict(consts)
        m["x"] = x[core * B_LOC:(core + 1) * B_LOC]
        in_maps.append(m)
    trace = bool(int(os.environ.get("TRN_KERNEL_TRACE", "0")))
    if trace:
        try:
            from antenv.axon_hooks import get_axon_ntff_profile_hook
            trace = get_axon_ntff_profile_hook() is not None
        except ImportError:
            trace = False
    res = run_bass_kernel_spmd(nc, in_maps, list(range(NCORES)), trace=trace)
    LAST_RESULTS = res
    out_w = np.concatenate([res.results[i]["out_w"] for i in range(NCORES)], axis=0)
    out_h = np.concatenate([res.results[i]["out_h"] for i in range(NCORES)], axis=0)
    return out_w, out_h
